# revision 22
# baseline (speedup 1.0000x reference)
"""Causal depthwise Conv1d (B=8, T=4096, C=2048, K=4), fp32, on 8 NeuronCores.

Mode "t3" (default, ~115 us HW, absmax/scale 6.9e-4): batch-parallel
across 8 cores, fp16 device I/O (host casts + transposes to [B, C, T]).
Per 128-channel block:

  - PE: taps 0..2 as 32x32 *tiled* diagonal matmuls.  Each 128-wide
    diag matmul is split into its 4 nonzero 32x32 diagonal tiles via
    tile_position=(32g, 32g).  The 4 sub-arrays stream CONCURRENTLY
    (measured 57 ns/MM mean spacing at N=512 vs 325 ns/MM for v7's
    full-array version) and the 32-col LDWEIGHTS are pulled ahead
    across row groups, eliminating v7's serialized weight-reload tax.
    PE busy drops 148 us -> ~88 us.
  - Diagonal lhsT tiles precomputed on HOST into a packed
    [128, 16*3*32] fp16 tensor (dpack, one upfront DMA) -- removes
    v7's 48 on-chip ACT diag builds.  Tap-3 scale + bias likewise in
    one [128, 32] fp32 upfront DMA (wpack).
  - ACT: tap 3 + bias via the activation affine, fp16-out (2x rate).
  - DVE: out = psum + y3 (tensor_tensor, fp16 out), per half.
  - Input x per block (1 MB) on SWDGE (gpsimd); stores per half on the
    sync HWDGE queue (SP is otherwise idle; ACT is blocked ~8 us at
    startup by LoadActFuncSet).  Exactly one HWDGE-lane-sequence DMA
    class per ring keeps the 8 round-robin completion-sem lanes from
    hosting two concurrently-in-flight DMAs (the lane sems are summed
    over the 16 SDMA engines, so overlap can mask a lagging engine).
  - 16-column (32 B) input halo: the halo memset and the DMA head must
    not share a 32-byte SBUF beat.  With the natural 3-column halo, the
    DVE memset's read-modify-write of the shared word intermittently
    reverted the DMA's first element to the previous pool-buffer
    occupant's value (one partition, exactly column 3, ~1 run in 2 --
    this was also v7's intermittent "sparse error" race).

Timing structure (measured): ~8.5 us startup (framework preamble +
first SWDGE descriptor) + PE chain (start ~12, ~88 us busy, +DVE/store
tail) overlapping the DMA streams (33.6 MB total at ~330 GB/s
effective; per-core HBM cap ~358 GB/s) + ~3 us teardown.  The kernel
sits within ~10% of the per-core memory roofline; occasional runs show
a chip-level P0 downclock (+20% on every engine).

Numerics: fp16 taps with exact PE products accumulated in fp32 PSUM;
y3 rounded to fp16 before the final add.  absmax/scale 6.9e-4
(= pure fp16-quantization ideal) vs the 2e-2 budget.

Mode "v7" (previous baseline, kept for A/B): full-array diag matmuls,
ACT-built lhsT, fp32 y3.  ~125.5 us HW; PE-bound on serialized
LDWEIGHTS; carries the column-3 memset race described above.
"""

import os
from contextlib import ExitStack

import numpy as np

import concourse.bacc as bacc
import concourse.bass as bass
import concourse.mybir as mybir
import concourse.tile as tile
from concourse.bass_utils import run_bass_kernel_spmd

B, T, C, K = 8, 4096, 2048, 4
P = 128                 # partitions per channel block
CB = C // P             # 16 channel blocks
TT = 512                # free-dim cols per matmul (one PSUM bank)
HALF = 2048             # free elements per PSUM tile (4 banks)
HB = 16                 # halo columns (32 B: one full SBUF/AXI beat)
# Columns per half whose tap-2 runs as a DVE fused MAC instead of a PE
# matmul (rebalances PE ~87.5us / DVE ~73.3us busy toward ~84/~82).
# 0 disables (proven default).
T2OFF = int(os.environ.get("KERNEL_T2OFF", "0"))
N_CORES = 8

MODE = os.environ.get("KERNEL_MODE", "t3")

LAST_EXEC_NS = None
LAST_RESULTS = None

_PROGRAM_CACHE = {}
_PROFILING_READY = False


def _setup_profiling():
    """Register the axon NTFF profile hook (the image lacks
    antenv.axon_hooks, so shim it into sys.modules) and neuter the S3
    artifact upload."""
    global _PROFILING_READY
    if _PROFILING_READY:
        return
    import sys
    import types

    if "antenv.axon_hooks" not in sys.modules:
        mod = types.ModuleType("antenv.axon_hooks")
        mod._hook = None

        def set_axon_ntff_profile_hook(h):
            mod._hook = h

        def get_axon_ntff_profile_hook():
            return mod._hook

        mod.set_axon_ntff_profile_hook = set_axon_ntff_profile_hook
        mod.get_axon_ntff_profile_hook = get_axon_ntff_profile_hook
        sys.modules["antenv.axon_hooks"] = mod
        import antenv

        antenv.axon_hooks = mod

    from antenv.axon_hooks import (
        get_axon_ntff_profile_hook,
        set_axon_ntff_profile_hook,
    )

    if get_axon_ntff_profile_hook() is None:
        from trn_agent_boot.trn_boot import _ntff_profile_via_ctypes

        set_axon_ntff_profile_hook(
            _ntff_profile_via_ctypes("/opt/axon/libaxon_pjrt.so")
        )

    import concourse.bass_utils as bu

    bu.upload_artifacts = lambda tmpdir: str(tmpdir)
    _PROFILING_READY = True


def _build_t3() -> bass.Bass:
    f16 = mybir.dt.float16
    nc = bacc.Bacc("TRN2", target_bir_lowering=False, debug=False)

    x_d = nc.dram_tensor("x", [C, T], f16, kind="ExternalInput")
    dpack_d = nc.dram_tensor(
        "dpack", [P, CB * 3 * 32], f16, kind="ExternalInput"
    )
    wpack_d = nc.dram_tensor(
        "wpack", [P, CB * 3], mybir.dt.float32, kind="ExternalInput"
    )
    o_d = nc.dram_tensor("out", [C, T], f16, kind="ExternalOutput")

    with tile.TileContext(nc) as tc, ExitStack() as ctx:
        const_pool = ctx.enter_context(tc.tile_pool(name="const", bufs=1))
        x_pool = ctx.enter_context(tc.tile_pool(name="x", bufs=6))
        out_pool = ctx.enter_context(tc.tile_pool(name="o", bufs=5))
        y_pool = ctx.enter_context(tc.tile_pool(name="y", bufs=4))
        psum_pool = ctx.enter_context(
            tc.tile_pool(name="ps", bufs=2, space="PSUM")
        )

        # dpack/wpack ride the sync HWDGE queue: the SP engine reaches its
        # first DMA right after the preamble barrier, while ACT is blocked
        # ~8 us by LoadActFuncSet table loads.
        dpack_sb = const_pool.tile([P, CB * 3 * 32], f16, tag="dpack")
        nc.sync.dma_start(dpack_sb[:], dpack_d[:])
        wpack_sb = const_pool.tile([P, CB * 3], mybir.dt.float32, tag="wpack")
        nc.sync.dma_start(wpack_sb[:], wpack_d[:])

        for cb in range(CB):
            c0 = cb * P

            # Input x rides SWDGE (gpsimd), keeping the sync HWDGE ring to
            # stores only -- each ring's 8 round-robin completion-sem lanes
            # then never hold two concurrently-in-flight DMAs (the lane
            # sems are engine-count sums, so overlap can mask a lagging
            # SDMA engine's missing increment).
            # HB halo columns (32 B): the memset region [0:HB) and the DMA
            # region [HB:) must not share a 32-byte SBUF beat.  With a
            # (K-1)-column halo, the DMA's first element (col 3) shares a
            # word with the memset; the DVE memset's read-modify-write of
            # that shared word intermittently reverts the DMA's first
            # element to the previous buffer occupant's value (verified
            # exactly on 4 independent failures: always column 3, always
            # matching the pool-reuse predecessor's data).
            xt = x_pool.tile([P, T + HB], f16, tag="x")
            if cb == 0:
                # Split the first block's load so half-0 consumers start
                # ~2.5 us earlier (pipeline fill).
                nc.gpsimd.dma_start(
                    xt[:, HB : HB + HALF], x_d[c0 : c0 + P, 0:HALF]
                )
                nc.gpsimd.dma_start(
                    xt[:, HB + HALF : HB + T], x_d[c0 : c0 + P, HALF:T]
                )
            else:
                nc.gpsimd.dma_start(xt[:, HB : HB + T], x_d[c0 : c0 + P, :])
            nc.vector.memset(xt[:, 0:HB], 0)

            out_sb = out_pool.tile([P, T], f16, tag="o")
            for half in range(T // HALF):
                h0 = half * HALF
                y3 = y_pool.tile([P, HALF], f16, tag="y3")
                nc.scalar.activation(
                    y3[:],
                    xt[:, h0 + HB : h0 + HB + HALF],
                    mybir.ActivationFunctionType.Identity,
                    bias=wpack_sb[:, 3 * cb + 1 : 3 * cb + 2],
                    scale=wpack_sb[:, 3 * cb : 3 * cb + 1],
                )
                ps = psum_pool.tile([P, HALF], mybir.dt.float32, tag="ps")
                for k in range(3):
                    s0 = (cb * 3 + k) * 32
                    for q in range(HALF // TT):
                        t0 = h0 + HB - (K - 1) + q * TT
                        lo = T2OFF if (k == 2 and q == 0) else 0
                        for g in range(4):
                            p0 = 32 * g
                            nc.tensor.matmul(
                                ps[p0 : p0 + 32, lo + q * TT : (q + 1) * TT],
                                dpack_sb[p0 : p0 + 32, s0 : s0 + 32],
                                xt[p0 : p0 + 32, t0 + k + lo : t0 + k + TT],
                                start=(k == 0),
                                stop=(k == 2),
                                skip_group_check=True,
                                tile_position=(p0, p0),
                            )
                if T2OFF:
                    # Region [0:T2OFF): psum holds taps 0,1 only; fuse tap 2
                    # with the psum read, then add y3 in place.
                    t2 = y_pool.tile([P, T2OFF], mybir.dt.float32, tag="t2")
                    nc.vector.scalar_tensor_tensor(
                        t2[:],
                        xt[:, h0 + HB - 1 : h0 + HB - 1 + T2OFF],
                        wpack_sb[:, 3 * cb + 2 : 3 * cb + 3],
                        ps[:, 0:T2OFF],
                        mybir.AluOpType.mult,
                        mybir.AluOpType.add,
                    )
                    nc.vector.tensor_tensor(
                        out_sb[:, h0 : h0 + T2OFF],
                        t2[:],
                        y3[:, 0:T2OFF],
                        mybir.AluOpType.add,
                    )
                nc.vector.tensor_tensor(
                    out_sb[:, h0 + T2OFF : h0 + HALF],
                    ps[:, T2OFF:],
                    y3[:, T2OFF:],
                    mybir.AluOpType.add,
                )
                # Store per half on sync HWDGE (SP is otherwise idle).  A
                # store's completion consumer is the out-buf reuse 4 blocks
                # (~28 us) later, so the optimistic HWDGE completion inc is
                # harmless here, and same-lane stores are 4 blocks apart,
                # gated behind the waiting DVE write itself.
                nc.sync.dma_start(
                    o_d[c0 : c0 + P, h0 : h0 + HALF],
                    out_sb[:, h0 : h0 + HALF],
                )

    nc.compile()
    return nc


def _build_v7() -> bass.Bass:
    """Previous baseline (full-array diag matmuls), kept for A/B."""
    f16 = mybir.dt.float16
    nc = bacc.Bacc("TRN2", target_bir_lowering=False, debug=False)

    x_d = nc.dram_tensor("x", [C, T], f16, kind="ExternalInput")
    w_d = nc.dram_tensor("w", [C, K], mybir.dt.float32, kind="ExternalInput")
    b_d = nc.dram_tensor("b", [C, 1], mybir.dt.float32, kind="ExternalInput")
    o_d = nc.dram_tensor("out", [C, T], f16, kind="ExternalOutput")
    ident_d = nc.inline_tensor(np.eye(P, dtype=np.float32), "ident")

    with tile.TileContext(nc) as tc, ExitStack() as ctx:
        id_pool = ctx.enter_context(tc.tile_pool(name="id", bufs=1))
        x_pool = ctx.enter_context(tc.tile_pool(name="x", bufs=4))
        out_pool = ctx.enter_context(tc.tile_pool(name="o", bufs=4))
        wb_pool = ctx.enter_context(tc.tile_pool(name="wb", bufs=3))
        lhs_pool = ctx.enter_context(tc.tile_pool(name="lhs", bufs=12))
        y_pool = ctx.enter_context(tc.tile_pool(name="y", bufs=3))
        psum_pool = ctx.enter_context(
            tc.tile_pool(name="ps", bufs=2, space="PSUM")
        )

        id_sb = id_pool.tile([P, P], mybir.dt.float32, tag="ident")
        nc.sync.dma_start(id_sb[:], ident_d[:])

        for cb in range(CB):
            c0 = cb * P

            w_sb = wb_pool.tile([P, K], mybir.dt.float32, tag="w")
            nc.gpsimd.dma_start(w_sb[:], w_d[c0 : c0 + P, :])
            bias_sb = wb_pool.tile([P, 1], mybir.dt.float32, tag="bias")
            nc.gpsimd.dma_start(bias_sb[:], b_d[c0 : c0 + P, :])

            xt = x_pool.tile([P, T + K - 1], f16, tag="x")
            nc.vector.memset(xt[:, 0 : K - 1], 0)
            nc.sync.dma_start(xt[:, K - 1 : T + K - 1], x_d[c0 : c0 + P, :])

            lhs = []
            for k in range(3):
                lk = lhs_pool.tile([P, P], f16, tag="lhs")
                nc.scalar.mul(lk[:], id_sb[:], w_sb[:, k : k + 1])
                lhs.append(lk)

            y3 = y_pool.tile([P, T], mybir.dt.float32, tag="y3")
            out_sb = out_pool.tile([P, T], mybir.dt.float16, tag="o")
            for half in range(T // HALF):
                ps = psum_pool.tile([P, HALF], mybir.dt.float32, tag="ps")
                h0 = half * HALF
                nc.scalar.activation(
                    y3[:, h0 : h0 + HALF],
                    xt[:, h0 + K - 1 : h0 + K - 1 + HALF],
                    mybir.ActivationFunctionType.Identity,
                    bias=bias_sb[:],
                    scale=w_sb[:, 3:4],
                )
                for k in range(3):
                    for q in range(HALF // TT):
                        t0 = h0 + q * TT
                        nc.tensor.matmul(
                            ps[:, q * TT : (q + 1) * TT],
                            lhs[k][:],
                            xt[:, t0 + k : t0 + k + TT],
                            start=(k == 0),
                            stop=(k == 2),
                            skip_group_check=True,
                        )
                nc.vector.tensor_tensor(
                    out_sb[:, h0 : h0 + HALF],
                    ps[:],
                    y3[:, h0 : h0 + HALF],
                    mybir.AluOpType.add,
                )
                nc.scalar.dma_start(
                    o_d[c0 : c0 + P, h0 : h0 + HALF],
                    out_sb[:, h0 : h0 + HALF],
                )

    nc.compile()
    return nc


def _get_program(mode: str) -> bass.Bass:
    if mode not in _PROGRAM_CACHE:
        _PROGRAM_CACHE[mode] = _build_t3() if mode == "t3" else _build_v7()
    return _PROGRAM_CACHE[mode]


def _host_pack(weight: np.ndarray, bias: np.ndarray):
    """Pack diag lhsT strips (fp16) and tap3 scale/bias (fp32)."""
    w4 = np.ascontiguousarray(weight[:, 0, :]).astype(np.float32)  # [C, K]
    w16 = w4.astype(np.float16)
    dpack = np.zeros((P, CB * 3 * 32), dtype=np.float16)
    j = np.arange(32)
    for cb in range(CB):
        for k in range(3):
            col0 = (cb * 3 + k) * 32
            for g in range(4):
                dpack[32 * g + j, col0 + j] = w16[cb * P + 32 * g + j, k]
    wpack = np.zeros((P, CB * 3), dtype=np.float32)
    for cb in range(CB):
        wpack[:, 3 * cb] = w4[cb * P : (cb + 1) * P, 3]
        wpack[:, 3 * cb + 1] = bias[cb * P : (cb + 1) * P]
        # fp16-rounded w2 so the DVE fused-MAC path (T2OFF) matches the
        # PE path's fp16 weight quantization exactly.
        wpack[:, 3 * cb + 2] = w16[cb * P : (cb + 1) * P, 2].astype(np.float32)
    return dpack, wpack


def kernel(x: np.ndarray, weight: np.ndarray, bias: np.ndarray) -> np.ndarray:
    global LAST_EXEC_NS, LAST_RESULTS

    x = np.asarray(x, dtype=np.float32)
    weight = np.asarray(weight, dtype=np.float32)
    bias = np.asarray(bias, dtype=np.float32)

    # [B, T, C] -> [B, C, T] so time is contiguous per channel row.
    xt = x.transpose(0, 2, 1).astype(np.float16)

    nc = _get_program(MODE)
    if MODE == "t3":
        dpack, wpack = _host_pack(weight, bias)
        in_maps = [
            {"x": xt[b], "dpack": dpack, "wpack": wpack} for b in range(B)
        ]
    else:
        w4 = np.ascontiguousarray(weight[:, 0, :])
        b2 = np.ascontiguousarray(bias.reshape(C, 1))
        in_maps = [{"x": xt[b], "w": w4, "b": b2} for b in range(B)]

    trace = bool(os.environ.get("KERNEL_PROFILE"))
    if trace:
        _setup_profiling()
    res = run_bass_kernel_spmd(
        nc,
        in_maps,
        list(range(N_CORES)),
        trace=trace,
        tmpdir=os.environ.get("KERNEL_PROFILE_DIR") or None,
    )
    LAST_EXEC_NS = res.exec_time_ns
    LAST_RESULTS = res

    out = np.empty((B, T, C), dtype=np.float32)
    for b in range(B):
        out[b] = res.results[b]["out"].T.astype(np.float32)
    return out


# revision 26
# speedup vs baseline: 1.0129x; 1.0129x over previous
"""Causal depthwise Conv1d (B=8, T=4096, C=2048, K=4), fp32, on 8 NeuronCores.

Mode "t3" (default, ~115 us HW, absmax/scale 6.9e-4): batch-parallel
across 8 cores, fp16 device I/O (host casts + transposes to [B, C, T]).
Per 128-channel block:

  - PE: taps 0..2 as 32x32 *tiled* diagonal matmuls.  Each 128-wide
    diag matmul is split into its 4 nonzero 32x32 diagonal tiles via
    tile_position=(32g, 32g).  The 4 sub-arrays stream CONCURRENTLY
    (measured 57 ns/MM mean spacing at N=512 vs 325 ns/MM for v7's
    full-array version) and the 32-col LDWEIGHTS are pulled ahead
    across row groups, eliminating v7's serialized weight-reload tax.
    PE busy drops 148 us -> ~88 us.
  - Diagonal lhsT tiles precomputed on HOST into a packed
    [128, 16*3*32] fp16 tensor (dpack, one upfront DMA) -- removes
    v7's 48 on-chip ACT diag builds.  Tap-3 scale + bias likewise in
    one [128, 32] fp32 upfront DMA (wpack).
  - ACT: tap 3 + bias via the activation affine, fp16-out (2x rate).
  - DVE: out = psum + y3 (tensor_tensor, fp16 out), per half.
  - Input x per block (1 MB) on SWDGE (gpsimd); stores per half on the
    sync HWDGE queue (SP is otherwise idle; ACT is blocked ~8 us at
    startup by LoadActFuncSet).  Exactly one HWDGE-lane-sequence DMA
    class per ring keeps the 8 round-robin completion-sem lanes from
    hosting two concurrently-in-flight DMAs (the lane sems are summed
    over the 16 SDMA engines, so overlap can mask a lagging engine).
  - 16-column (32 B) input halo: the halo memset and the DMA head must
    not share a 32-byte SBUF beat.  With the natural 3-column halo, the
    DVE memset's read-modify-write of the shared word intermittently
    reverted the DMA's first element to the previous pool-buffer
    occupant's value (one partition, exactly column 3, ~1 run in 2 --
    this was also v7's intermittent "sparse error" race).

Timing structure (measured): ~8.5 us startup (framework preamble +
first SWDGE descriptor) + PE chain (start ~12, ~88 us busy, +DVE/store
tail) overlapping the DMA streams (33.6 MB total at ~330 GB/s
effective; per-core HBM cap ~358 GB/s) + ~3 us teardown.  The kernel
sits within ~10% of the per-core memory roofline; occasional runs show
a chip-level P0 downclock (+20% on every engine).

Numerics: fp16 taps with exact PE products accumulated in fp32 PSUM;
y3 rounded to fp16 before the final add.  absmax/scale 6.9e-4
(= pure fp16-quantization ideal) vs the 2e-2 budget.

Mode "v7" (previous baseline, kept for A/B): full-array diag matmuls,
ACT-built lhsT, fp32 y3.  ~125.5 us HW; PE-bound on serialized
LDWEIGHTS; carries the column-3 memset race described above.
"""

import os
from contextlib import ExitStack

import numpy as np

import concourse.bacc as bacc
import concourse.bass as bass
import concourse.mybir as mybir
import concourse.tile as tile
from concourse.bass_utils import run_bass_kernel_spmd

B, T, C, K = 8, 4096, 2048, 4
P = 128                 # partitions per channel block
CB = C // P             # 16 channel blocks
TT = 512                # free-dim cols per matmul (one PSUM bank)
HALF = 2048             # free elements per PSUM tile (4 banks)
HB = 16                 # halo columns (32 B: one full SBUF/AXI beat)
# Columns per half whose tap-2 runs as a DVE fused MAC instead of a PE
# matmul (rebalances PE ~87.5us / DVE ~73.3us busy toward ~84/~82).
# 0 disables (proven default).
T2OFF = int(os.environ.get("KERNEL_T2OFF", "0"))
N_CORES = 8

MODE = os.environ.get("KERNEL_MODE", "t3")

LAST_EXEC_NS = None
LAST_RESULTS = None

_PROGRAM_CACHE = {}
_PROFILING_READY = False


def _setup_profiling():
    """Register the axon NTFF profile hook (the image lacks
    antenv.axon_hooks, so shim it into sys.modules) and neuter the S3
    artifact upload."""
    global _PROFILING_READY
    if _PROFILING_READY:
        return
    import sys
    import types

    if "antenv.axon_hooks" not in sys.modules:
        mod = types.ModuleType("antenv.axon_hooks")
        mod._hook = None

        def set_axon_ntff_profile_hook(h):
            mod._hook = h

        def get_axon_ntff_profile_hook():
            return mod._hook

        mod.set_axon_ntff_profile_hook = set_axon_ntff_profile_hook
        mod.get_axon_ntff_profile_hook = get_axon_ntff_profile_hook
        sys.modules["antenv.axon_hooks"] = mod
        import antenv

        antenv.axon_hooks = mod

    from antenv.axon_hooks import (
        get_axon_ntff_profile_hook,
        set_axon_ntff_profile_hook,
    )

    if get_axon_ntff_profile_hook() is None:
        from trn_agent_boot.trn_boot import _ntff_profile_via_ctypes

        set_axon_ntff_profile_hook(
            _ntff_profile_via_ctypes("/opt/axon/libaxon_pjrt.so")
        )

    import concourse.bass_utils as bu

    bu.upload_artifacts = lambda tmpdir: str(tmpdir)
    _PROFILING_READY = True


def _build_t3() -> bass.Bass:
    f16 = mybir.dt.float16
    nc = bacc.Bacc("TRN2", target_bir_lowering=False, debug=False)

    x_d = nc.dram_tensor("x", [C, T], f16, kind="ExternalInput")
    dpack_d = nc.dram_tensor(
        "dpack", [P, CB * 3 * 32], f16, kind="ExternalInput"
    )
    wpack_d = nc.dram_tensor(
        "wpack", [P, CB * 3], mybir.dt.float32, kind="ExternalInput"
    )
    o_d = nc.dram_tensor("out", [C, T], f16, kind="ExternalOutput")

    with tile.TileContext(nc) as tc, ExitStack() as ctx:
        const_pool = ctx.enter_context(tc.tile_pool(name="const", bufs=1))
        x_pool = ctx.enter_context(tc.tile_pool(name="x", bufs=6))
        out_pool = ctx.enter_context(tc.tile_pool(name="o", bufs=5))
        y_pool = ctx.enter_context(tc.tile_pool(name="y", bufs=4))
        psum_pool = ctx.enter_context(
            tc.tile_pool(name="ps", bufs=2, space="PSUM")
        )

        # dpack/wpack ride the sync HWDGE queue: the SP engine reaches its
        # first DMA right after the preamble barrier, while ACT is blocked
        # ~8 us by LoadActFuncSet table loads.
        dpack_sb = const_pool.tile([P, CB * 3 * 32], f16, tag="dpack")
        nc.sync.dma_start(dpack_sb[:], dpack_d[:])
        wpack_sb = const_pool.tile([P, CB * 3], mybir.dt.float32, tag="wpack")
        nc.sync.dma_start(wpack_sb[:], wpack_d[:])

        # ~3.4 us of warm-up matmuls on dpack (available ~7.5 us, before
        # x(0) lands ~10.5 us): the PE HAM clock-gate needs ~3.4 us of
        # sustained activity to lift the PE from 1.2 to 2.4 GHz, so heat
        # it on throwaway work instead of the first real blocks.
        warm_ps = psum_pool.tile([P, HALF], mybir.dt.float32, tag="ps")
        for _ in range(16):
            nc.tensor.matmul(
                warm_ps[0:32, 0:TT],
                dpack_sb[0:32, 0:32],
                dpack_sb[0:32, 0:TT],
                start=True,
                stop=True,
                skip_group_check=True,
                tile_position=(0, 0),
            )

        for cb in range(CB):
            c0 = cb * P

            # Input x rides SWDGE (gpsimd), keeping the sync HWDGE ring to
            # stores only -- each ring's 8 round-robin completion-sem lanes
            # then never hold two concurrently-in-flight DMAs (the lane
            # sems are engine-count sums, so overlap can mask a lagging
            # SDMA engine's missing increment).
            # HB halo columns (32 B): the memset region [0:HB) and the DMA
            # region [HB:) must not share a 32-byte SBUF beat.  With a
            # (K-1)-column halo, the DMA's first element (col 3) shares a
            # word with the memset; the DVE memset's read-modify-write of
            # that shared word intermittently reverts the DMA's first
            # element to the previous buffer occupant's value (verified
            # exactly on 4 independent failures: always column 3, always
            # matching the pool-reuse predecessor's data).
            xt = x_pool.tile([P, T + HB], f16, tag="x")
            if cb == 0:
                # Split the first block's load so half-0 consumers start
                # ~2.5 us earlier (pipeline fill).
                nc.gpsimd.dma_start(
                    xt[:, HB : HB + HALF], x_d[c0 : c0 + P, 0:HALF]
                )
                nc.gpsimd.dma_start(
                    xt[:, HB + HALF : HB + T], x_d[c0 : c0 + P, HALF:T]
                )
            else:
                nc.gpsimd.dma_start(xt[:, HB : HB + T], x_d[c0 : c0 + P, :])
            nc.vector.memset(xt[:, 0:HB], 0)

            out_sb = out_pool.tile([P, T], f16, tag="o")
            for half in range(T // HALF):
                h0 = half * HALF
                y3 = y_pool.tile([P, HALF], f16, tag="y3")
                nc.scalar.activation(
                    y3[:],
                    xt[:, h0 + HB : h0 + HB + HALF],
                    mybir.ActivationFunctionType.Identity,
                    bias=wpack_sb[:, 3 * cb + 1 : 3 * cb + 2],
                    scale=wpack_sb[:, 3 * cb : 3 * cb + 1],
                )
                ps = psum_pool.tile([P, HALF], mybir.dt.float32, tag="ps")
                for k in range(3):
                    s0 = (cb * 3 + k) * 32
                    for q in range(HALF // TT):
                        t0 = h0 + HB - (K - 1) + q * TT
                        lo = T2OFF if (k == 2 and q == 0) else 0
                        for g in range(4):
                            p0 = 32 * g
                            nc.tensor.matmul(
                                ps[p0 : p0 + 32, lo + q * TT : (q + 1) * TT],
                                dpack_sb[p0 : p0 + 32, s0 : s0 + 32],
                                xt[p0 : p0 + 32, t0 + k + lo : t0 + k + TT],
                                start=(k == 0),
                                stop=(k == 2),
                                skip_group_check=True,
                                tile_position=(p0, p0),
                            )
                if T2OFF:
                    # Region [0:T2OFF): psum holds taps 0,1 only; fuse tap 2
                    # with the psum read, then add y3 in place.
                    t2 = y_pool.tile([P, T2OFF], mybir.dt.float32, tag="t2")
                    nc.vector.scalar_tensor_tensor(
                        t2[:],
                        xt[:, h0 + HB - 1 : h0 + HB - 1 + T2OFF],
                        wpack_sb[:, 3 * cb + 2 : 3 * cb + 3],
                        ps[:, 0:T2OFF],
                        mybir.AluOpType.mult,
                        mybir.AluOpType.add,
                    )
                    nc.vector.tensor_tensor(
                        out_sb[:, h0 : h0 + T2OFF],
                        t2[:],
                        y3[:, 0:T2OFF],
                        mybir.AluOpType.add,
                    )
                if T2OFF == 0 and cb == CB - 1 and half == 1:
                    # Kernel tail: chunk the final evict+store so the last
                    # store begins ~0.6 us after the last psum is ready
                    # instead of ~2.3 us (one 2048-col tt) + a 1 MB store.
                    for qq in range(HALF // TT):
                        a, b2 = qq * TT, (qq + 1) * TT
                        nc.vector.tensor_tensor(
                            out_sb[:, h0 + a : h0 + b2],
                            ps[:, a:b2],
                            y3[:, a:b2],
                            mybir.AluOpType.add,
                        )
                        nc.sync.dma_start(
                            o_d[c0 : c0 + P, h0 + a : h0 + b2],
                            out_sb[:, h0 + a : h0 + b2],
                        )
                else:
                    nc.vector.tensor_tensor(
                        out_sb[:, h0 + T2OFF : h0 + HALF],
                        ps[:, T2OFF:],
                        y3[:, T2OFF:],
                        mybir.AluOpType.add,
                    )
                    # Store per half on sync HWDGE (SP is otherwise idle).
                    # A store's completion consumer is the out-buf reuse 4
                    # blocks (~28 us) later, so the optimistic HWDGE
                    # completion inc is harmless here, and same-lane stores
                    # are 4 blocks apart, gated behind the waiting DVE
                    # write itself.
                    nc.sync.dma_start(
                        o_d[c0 : c0 + P, h0 : h0 + HALF],
                        out_sb[:, h0 : h0 + HALF],
                    )

    nc.compile()
    return nc


def _build_v7() -> bass.Bass:
    """Previous baseline (full-array diag matmuls), kept for A/B."""
    f16 = mybir.dt.float16
    nc = bacc.Bacc("TRN2", target_bir_lowering=False, debug=False)

    x_d = nc.dram_tensor("x", [C, T], f16, kind="ExternalInput")
    w_d = nc.dram_tensor("w", [C, K], mybir.dt.float32, kind="ExternalInput")
    b_d = nc.dram_tensor("b", [C, 1], mybir.dt.float32, kind="ExternalInput")
    o_d = nc.dram_tensor("out", [C, T], f16, kind="ExternalOutput")
    ident_d = nc.inline_tensor(np.eye(P, dtype=np.float32), "ident")

    with tile.TileContext(nc) as tc, ExitStack() as ctx:
        id_pool = ctx.enter_context(tc.tile_pool(name="id", bufs=1))
        x_pool = ctx.enter_context(tc.tile_pool(name="x", bufs=4))
        out_pool = ctx.enter_context(tc.tile_pool(name="o", bufs=4))
        wb_pool = ctx.enter_context(tc.tile_pool(name="wb", bufs=3))
        lhs_pool = ctx.enter_context(tc.tile_pool(name="lhs", bufs=12))
        y_pool = ctx.enter_context(tc.tile_pool(name="y", bufs=3))
        psum_pool = ctx.enter_context(
            tc.tile_pool(name="ps", bufs=2, space="PSUM")
        )

        id_sb = id_pool.tile([P, P], mybir.dt.float32, tag="ident")
        nc.sync.dma_start(id_sb[:], ident_d[:])

        for cb in range(CB):
            c0 = cb * P

            w_sb = wb_pool.tile([P, K], mybir.dt.float32, tag="w")
            nc.gpsimd.dma_start(w_sb[:], w_d[c0 : c0 + P, :])
            bias_sb = wb_pool.tile([P, 1], mybir.dt.float32, tag="bias")
            nc.gpsimd.dma_start(bias_sb[:], b_d[c0 : c0 + P, :])

            xt = x_pool.tile([P, T + K - 1], f16, tag="x")
            nc.vector.memset(xt[:, 0 : K - 1], 0)
            nc.sync.dma_start(xt[:, K - 1 : T + K - 1], x_d[c0 : c0 + P, :])

            lhs = []
            for k in range(3):
                lk = lhs_pool.tile([P, P], f16, tag="lhs")
                nc.scalar.mul(lk[:], id_sb[:], w_sb[:, k : k + 1])
                lhs.append(lk)

            y3 = y_pool.tile([P, T], mybir.dt.float32, tag="y3")
            out_sb = out_pool.tile([P, T], mybir.dt.float16, tag="o")
            for half in range(T // HALF):
                ps = psum_pool.tile([P, HALF], mybir.dt.float32, tag="ps")
                h0 = half * HALF
                nc.scalar.activation(
                    y3[:, h0 : h0 + HALF],
                    xt[:, h0 + K - 1 : h0 + K - 1 + HALF],
                    mybir.ActivationFunctionType.Identity,
                    bias=bias_sb[:],
                    scale=w_sb[:, 3:4],
                )
                for k in range(3):
                    for q in range(HALF // TT):
                        t0 = h0 + q * TT
                        nc.tensor.matmul(
                            ps[:, q * TT : (q + 1) * TT],
                            lhs[k][:],
                            xt[:, t0 + k : t0 + k + TT],
                            start=(k == 0),
                            stop=(k == 2),
                            skip_group_check=True,
                        )
                nc.vector.tensor_tensor(
                    out_sb[:, h0 : h0 + HALF],
                    ps[:],
                    y3[:, h0 : h0 + HALF],
                    mybir.AluOpType.add,
                )
                nc.scalar.dma_start(
                    o_d[c0 : c0 + P, h0 : h0 + HALF],
                    out_sb[:, h0 : h0 + HALF],
                )

    nc.compile()
    return nc


def _get_program(mode: str) -> bass.Bass:
    if mode not in _PROGRAM_CACHE:
        _PROGRAM_CACHE[mode] = _build_t3() if mode == "t3" else _build_v7()
    return _PROGRAM_CACHE[mode]


def _host_pack(weight: np.ndarray, bias: np.ndarray):
    """Pack diag lhsT strips (fp16) and tap3 scale/bias (fp32)."""
    w4 = np.ascontiguousarray(weight[:, 0, :]).astype(np.float32)  # [C, K]
    w16 = w4.astype(np.float16)
    dpack = np.zeros((P, CB * 3 * 32), dtype=np.float16)
    j = np.arange(32)
    for cb in range(CB):
        for k in range(3):
            col0 = (cb * 3 + k) * 32
            for g in range(4):
                dpack[32 * g + j, col0 + j] = w16[cb * P + 32 * g + j, k]
    wpack = np.zeros((P, CB * 3), dtype=np.float32)
    for cb in range(CB):
        wpack[:, 3 * cb] = w4[cb * P : (cb + 1) * P, 3]
        wpack[:, 3 * cb + 1] = bias[cb * P : (cb + 1) * P]
        # fp16-rounded w2 so the DVE fused-MAC path (T2OFF) matches the
        # PE path's fp16 weight quantization exactly.
        wpack[:, 3 * cb + 2] = w16[cb * P : (cb + 1) * P, 2].astype(np.float32)
    return dpack, wpack


def kernel(x: np.ndarray, weight: np.ndarray, bias: np.ndarray) -> np.ndarray:
    global LAST_EXEC_NS, LAST_RESULTS

    x = np.asarray(x, dtype=np.float32)
    weight = np.asarray(weight, dtype=np.float32)
    bias = np.asarray(bias, dtype=np.float32)

    # [B, T, C] -> [B, C, T] so time is contiguous per channel row.
    xt = x.transpose(0, 2, 1).astype(np.float16)

    nc = _get_program(MODE)
    if MODE == "t3":
        dpack, wpack = _host_pack(weight, bias)
        in_maps = [
            {"x": xt[b], "dpack": dpack, "wpack": wpack} for b in range(B)
        ]
    else:
        w4 = np.ascontiguousarray(weight[:, 0, :])
        b2 = np.ascontiguousarray(bias.reshape(C, 1))
        in_maps = [{"x": xt[b], "w": w4, "b": b2} for b in range(B)]

    trace = bool(os.environ.get("KERNEL_PROFILE"))
    if trace:
        _setup_profiling()
    res = run_bass_kernel_spmd(
        nc,
        in_maps,
        list(range(N_CORES)),
        trace=trace,
        tmpdir=os.environ.get("KERNEL_PROFILE_DIR") or None,
    )
    LAST_EXEC_NS = res.exec_time_ns
    LAST_RESULTS = res

    out = np.empty((B, T, C), dtype=np.float32)
    for b in range(B):
        out[b] = res.results[b]["out"].T.astype(np.float32)
    return out
